# revision 1
# baseline (speedup 1.0000x reference)
"""Trainium2 Bass kernel for windowed (block-diagonal) multi-head video attention.

Problem: x:[2,8192,1024] -> qkv proj -> 3D-window (2,8,8) attention over a
(8,32,32) token grid, 16 heads x 64 dim -> out proj -> [2,8192,1024].

Sharding: 8 cores, data-parallel over (batch, t-window-group).  Token order is
(t,h,w)-major, so the slab x[b, it*2048:(it+1)*2048, :] is contiguous and holds
exactly the 16 independent (h,w)-windows with t in {2it, 2it+1}.  Each core:
  - DMA-gathers each window's 128 tokens as a [128,1024] tile (strided AP)
  - PE-transposes x_win -> x^T (contraction dim on partitions)
  - QKV projection: Q,K produced head-transposed [oc,tok]; V token-major with a
    per-head ones column appended (65-stride layout)
  - S^T = K_h Q_h^T per head (K=64), exp on ACT, A·V matmul where the ones row
    yields the softmax denominator for free; normalize with reciprocal +
    gpsimd partition-broadcast + DVE multiply
  - out projection, DMA-scatter back to token order
Weights are pre-transposed on the host; biases (zero in this problem) are
supported via rank-1 (K=1) accumulation matmuls, compiled only when nonzero.
"""

import sys

for _p in ("/opt/trn_rl_repo",):
    if _p not in sys.path:
        sys.path.insert(0, _p)

import numpy as np

B, T, H, W = 2, 8, 32, 32
C, NH, HD = 1024, 16, 64
WT, WH, WW = 2, 8, 8
N = T * H * W              # 8192 tokens
SCALE = HD ** -0.5
NCORES = 8
SLAB = N // (T // WT)      # 2048 tokens per (b, it) slab
NWIN = (H // WH) * (W // WW)   # 16 windows per slab
M = WT * WH * WW           # 128 tokens per window
KC = C // 128              # 8 contraction chunks

_BUILD_CACHE = {}


def _split_drain_waits(nc, mybir, cap=1, event_cap=2):
    """This walrus build accepts only one sem wait per TPB instruction
    (Tile's scheduler attaches up to 3).  Move the excess onto
    InstEventSemaphore carriers (which hold 2) inserted right before the
    over-subscribed instruction on the same engine — the engine blocks on the
    carriers first, so semantics are unchanged."""
    for f in nc.m.functions:
        for bb in f.blocks:
            i = 0
            while i < len(bb.instructions):
                ins = bb.instructions[i]
                si = ins.sync_info
                my_cap = (
                    event_cap
                    if type(ins).__name__ == "InstEventSemaphore"
                    else cap
                )
                if si is not None and si.on_wait and len(si.on_wait) > my_cap:
                    waits = list(si.on_wait)
                    si.on_wait = waits[:my_cap]
                    extra = waits[my_cap:]
                    carriers = []
                    while extra:
                        chunk, extra = extra[:event_cap], extra[event_cap:]
                        ev = mybir.InstEventSemaphore(
                            name=f"I-{nc.next_id()}-waitsplit", ins=[], outs=[]
                        )
                        ev.engine = ins.engine
                        ev.sync_info = mybir.SyncInfo(
                            on_wait=list(chunk), on_update=[]
                        )
                        nc.register_instruction(ev)
                        carriers.append(ev)
                    bb.instructions[i:i] = carriers
                    i += len(carriers)
                i += 1


def _build(has_qkvb, has_projb, use_f32r=True):
    import concourse.bass as bass
    import concourse.tile as tile
    from concourse import mybir
    f32 = mybir.dt.float32
    fpr = mybir.dt.float32r if use_f32r else f32

    nc = bass.Bass("TRN2", target_bir_lowering=False, debug=False)
    xs = nc.dram_tensor("xs", [SLAB, C], fpr, kind="ExternalInput")
    # weight dtype f32r: same 4-byte layout, PE rounds on read (tf32-like)
    wqkvT = nc.dram_tensor("wqkvT", [C, 3 * C], fpr, kind="ExternalInput")
    projT = nc.dram_tensor("projT", [C, C], fpr, kind="ExternalInput")
    if has_qkvb:
        qkvb = nc.dram_tensor("qkvb", [1, 3 * C], fpr, kind="ExternalInput")
    if has_projb:
        projb = nc.dram_tensor("projb", [1, C], fpr, kind="ExternalInput")
    ident_d = nc.dram_tensor("ident", [128, 128], fpr, kind="ExternalInput")
    out = nc.dram_tensor("out", [SLAB, C], f32, kind="ExternalOutput")

    # window gather/scatter views: slab token idx = tt*1024 + hh*32 + ww in a
    # [2, (4,8), (4,8)] = (tt, ih hh, iw ww) decomposition; window = (ih, iw)
    xs_v = xs.ap().rearrange(
        "(tt ih hh iw ww) c -> ih iw tt hh ww c", tt=WT, ih=4, hh=WH, iw=4, ww=WW
    )
    out_v = out.ap().rearrange(
        "(tt ih hh iw ww) c -> ih iw tt hh ww c", tt=WT, ih=4, hh=WH, iw=4, ww=WW
    )

    # windows processed in pairs: tok dim = 256 so the f32r matmuls hit the
    # 1 cyc/row regime (ap_size >= 256); attention blocks stay per-window
    GW = 2
    TOKG = 128 * GW

    with tile.TileContext(nc) as tc:
        with (
            tc.tile_pool(name="wq", bufs=1) as wq_pool,
            tc.tile_pool(name="wp", bufs=1) as wp_pool,
            tc.tile_pool(name="const", bufs=1) as const_pool,
            tc.tile_pool(name="xw", bufs=2 if not (has_qkvb or has_projb) else 1) as xw_pool,
            tc.tile_pool(name="xT", bufs=1) as xT_pool,
            tc.tile_pool(name="qk", bufs=1) as qk_pool,
            tc.tile_pool(name="v65", bufs=1) as v_pool,
            tc.tile_pool(name="E", bufs=3 if not (has_qkvb or has_projb) else 2) as e_pool,
            tc.tile_pool(name="rR", bufs=2 if not (has_qkvb or has_projb) else 1) as r_pool,
            tc.tile_pool(name="owT", bufs=1) as ow_pool,
            tc.tile_pool(name="o", bufs=1) as o_pool,
            tc.tile_pool(name="psA", bufs=4, space="PSUM") as psA,
            tc.tile_pool(name="psB", bufs=4, space="PSUM") as psB,
        ):
            # identity via DMA: make_identity runs on GpSimd, whose cold
            # start would gate the first PE transpose
            ident = const_pool.tile([128, 128], fpr)
            nc.scalar.dma_start(ident[:], ident_d.ap())
            ones_col = const_pool.tile([128, GW * NH], f32)
            nc.vector.memset(ones_col[:], 1.0)
            ones64f = const_pool.tile([1, 64], f32)
            nc.vector.memset(ones64f[:], 1.0)
            ones64 = const_pool.tile([1, 64], fpr)
            nc.scalar.copy(ones64[:], ones64f[:])

            wq_sb = wq_pool.tile([128, KC, 3 * C], fpr)
            wq_src = wqkvT.ap().rearrange("(k p) o -> p k o", p=128)
            for k in range(KC):
                nc.sync.dma_start(
                    wq_sb[:, k : k + 1, 0 : 2 * C], wq_src[:, k : k + 1, 0 : 2 * C]
                )
            for k in range(KC):
                nc.sync.dma_start(
                    wq_sb[:, k : k + 1, 2 * C :], wq_src[:, k : k + 1, 2 * C :]
                )
            wp_sb = wp_pool.tile([128, KC, C], fpr)
            wp_src = projT.ap().rearrange("(k p) o -> p k o", p=128)
            for k in range(KC):
                nc.sync.dma_start(wp_sb[:, k : k + 1, :], wp_src[:, k : k + 1, :])
            if has_qkvb or has_projb:
                onesf = const_pool.tile([1, TOKG], f32)
                nc.vector.memset(onesf[:], 1.0)
                ones = const_pool.tile([1, TOKG], fpr)
                nc.scalar.copy(ones[:], onesf[:])
            if has_qkvb:
                qkvb_sb = const_pool.tile([1, 3 * C], fpr)
                nc.sync.dma_start(qkvb_sb[:], qkvb.ap())
            if has_projb:
                projb_sb = const_pool.tile([1, C], fpr)
                nc.sync.dma_start(projb_sb[:], projb.ap())

            for grp in range(NWIN // GW):
                wins = [(divmod(GW * grp + w, 4)) for w in range(GW)]

                # 1+2) per window: gather tokens, PE-transpose into the
                # group x^T tile [c-chunk partitions, (chunk, tok)] (f32r)
                xT = xT_pool.tile([128, KC, TOKG], fpr)
                for w, (ih, iw) in enumerate(wins):
                    xw = xw_pool.tile([128, C], fpr)
                    for tt in range(WT):
                        nc.scalar.dma_start(
                            xw[64 * tt : 64 * (tt + 1), :], xs_v[ih, iw, tt]
                        )
                    for tb in range(2):
                        ps = psA.tile([128, 512], fpr, tag="psA")
                        for j in range(4):
                            jj = 4 * tb + j
                            nc.tensor.transpose(
                                ps[:, 128 * j : 128 * (j + 1)],
                                xw[:, 128 * jj : 128 * (jj + 1)],
                                ident[:],
                            )
                        psv = ps[:].rearrange("p (c t) -> p c t", t=128)
                        with nc.allow_low_precision(reason="f32r eviction"):
                            nc.vector.tensor_copy(
                                xT[:].rearrange("p k (g t) -> p k g t", g=GW)[
                                    :, 4 * tb : 4 * tb + 4, w, :
                                ],
                                psv[:],
                            )

                # 3) Q,K head-transposed: psum bank [oc 128, tok 256] x2 chunks.
                # Evict to 64-partition per-head layout (slot 2c+parity) so S
                # matmuls never use partition-base-64 operands (mixing base-0
                # and base-64 matmul operands hangs trn2).  qkT is f32r so the
                # S matmuls run as a single (rounded) pass instead of fp32's
                # HI+LO pair.
                qkT = qk_pool.tile([64, 4 * KC, TOKG], fpr)
                qkTv = qkT[:].rearrange("p (s two) t -> p s two t", two=2)
                for bank in (0, 4, 1, 5, 2, 6, 3, 7):
                    ps = psA.tile([128, 512], f32, tag="psA")
                    for sub in range(2):
                        oc = 2 * bank + sub
                        for k in range(KC):
                            nc.tensor.matmul(
                                ps[:, TOKG * sub : TOKG * (sub + 1)],
                                wq_sb[:, k, 128 * oc : 128 * (oc + 1)],
                                xT[:, k, :],
                                start=(k == 0),
                                stop=(k == KC - 1 and not has_qkvb),
                            )
                        if has_qkvb:
                            nc.tensor.matmul(
                                ps[:, TOKG * sub : TOKG * (sub + 1)],
                                qkvb_sb[0:1, 128 * oc : 128 * (oc + 1)],
                                ones[0:1, 0:TOKG],
                                start=False,
                                stop=True,
                            )
                    sc = SCALE if bank < 4 else 1.0
                    psv = ps[:].rearrange("p (c t) -> p c t", t=TOKG)
                    with nc.allow_low_precision(reason="f32r eviction"):
                        nc.vector.tensor_scalar_mul(
                            qkTv[:, 2 * bank : 2 * bank + 2, 0, :],
                            psv[0:64, :, :],
                            sc,
                        )
                        nc.vector.tensor_scalar_mul(
                            qkTv[:, 2 * bank : 2 * bank + 2, 1, :],
                            psv[64:128, :, :],
                            sc,
                        )

                # 4) V token-major per window, ones column per head (stride 65)
                v65 = v_pool.tile([128, GW, NH, HD + 1], fpr)
                nc.scalar.copy(
                    v65[:, :, :, HD : HD + 1],
                    ones_col[:].rearrange("p (g h) -> p g h", g=GW)[:, :, :, None],
                )
                for w in range(GW):
                    for nk in range(2):
                        ps = psA.tile([128, 512], f32, tag="psA")
                        for half in range(2):
                            lo = 2 * C + 512 * nk + 256 * half
                            for k in range(KC):
                                nc.tensor.matmul(
                                    ps[:, 256 * half : 256 * (half + 1)],
                                    xT[:].rearrange(
                                        "p k (g t) -> p k g t", g=GW
                                    )[:, k, w, :],
                                    wq_sb[:, k, lo : lo + 256],
                                    start=(k == 0),
                                    stop=(k == KC - 1 and not has_qkvb),
                                )
                            if has_qkvb:
                                nc.tensor.matmul(
                                    ps[:, 256 * half : 256 * (half + 1)],
                                    ones[0:1, 0:128],
                                    qkvb_sb[0:1, lo : lo + 256],
                                    start=False,
                                    stop=True,
                                )
                        # one strided eviction for all 8 heads of this bank
                        nc.scalar.copy(
                            v65[:, w, 8 * nk : 8 * nk + 8, 0:HD],
                            ps[:].rearrange("p (h e) -> p h e", e=HD),
                        )

                # 5+6) attention per (4-head bank, window), then out projection
                for w, (ih, iw) in enumerate(wins):
                    owT = ow_pool.tile([128, KC, 128], fpr)
                    # all 4 S banks first so exp/AV overlap the S matmuls
                    psS_banks = []
                    for hb in range(4):
                        psS = psB.tile([128, 512], f32, tag="psB")
                        for m in range(4):
                            h = 4 * hb + m
                            # S^T[kt,qt] = (K_h^T).T @ Q_h^T, K=64, base 0
                            nc.tensor.matmul(
                                psS[:, 128 * m : 128 * (m + 1)],
                                qkT[:, NH + h, 128 * w : 128 * (w + 1)],
                                qkT[:, h, 128 * w : 128 * (w + 1)],
                                start=True,
                                stop=True,
                            )
                        psS_banks.append(psS)
                    for hb in range(4):
                        E = e_pool.tile([128, 512], fpr, tag="E")
                        with nc.allow_low_precision(reason="f32r attn weights"):
                            nc.scalar.activation(
                                E[:],
                                psS_banks[hb][:],
                                mybir.ActivationFunctionType.Exp,
                            )
                        psV = psA.tile([128, 512], f32, tag="psA")
                        for m in range(4):
                            h = 4 * hb + m
                            # rows 0..63 = V^T E (unnormalized), row 64 = denom
                            nc.tensor.matmul(
                                psV[0:65, 128 * m : 128 * (m + 1)],
                                v65[:, w, h, :],
                                E[:, 128 * m : 128 * (m + 1)],
                                start=True,
                                stop=True,
                            )
                        # softmax 1/denom as exp(-ln(den)) on the ACT
                        # tables (InstReciprocal costs ~9 cyc/elem/lane and
                        # the denom row is a single-partition [1,512]);
                        # then partition-broadcast via a K=1 matmul
                        L = r_pool.tile([1, 512], f32, tag="r")
                        nc.scalar.activation(
                            L[:], psV[64:65, :], mybir.ActivationFunctionType.Ln
                        )
                        r = r_pool.tile([1, 512], fpr, tag="r")
                        with nc.allow_low_precision(reason="f32r recip"):
                            nc.scalar.activation(
                                r[:],
                                L[:],
                                mybir.ActivationFunctionType.Exp,
                                scale=-1.0,
                            )
                        Rp = psA.tile([64, 512], f32, tag="psA")
                        for half in range(2):
                            nc.tensor.matmul(
                                Rp[:, 256 * half : 256 * (half + 1)],
                                ones64[:],
                                r[0:1, 256 * half : 256 * (half + 1)],
                                start=True,
                                stop=True,
                            )
                        R = r_pool.tile([64, 512], f32, tag="R")
                        nc.scalar.copy(R[:], Rp[:])
                        for m in range(4):
                            h = 4 * hb + m
                            po = (h % 2) * 64
                            nc.vector.tensor_tensor(
                                owT[po : po + 64, h // 2, :],
                                psV[0:64, 128 * m : 128 * (m + 1)],
                                R[:, 128 * m : 128 * (m + 1)],
                                op=mybir.AluOpType.mult,
                            )

                    otile = o_pool.tile([128, C], f32)
                    for nk in range(2):
                        ps = psA.tile([128, 512], f32, tag="psA")
                        for half in range(2):
                            lo = 512 * nk + 256 * half
                            for k in range(KC):
                                nc.tensor.matmul(
                                    ps[:, 256 * half : 256 * (half + 1)],
                                    owT[:, k, :],
                                    wp_sb[:, k, lo : lo + 256],
                                    start=(k == 0),
                                    stop=(k == KC - 1 and not has_projb),
                                )
                            if has_projb:
                                nc.tensor.matmul(
                                    ps[:, 256 * half : 256 * (half + 1)],
                                    ones[0:1, 0:128],
                                    projb_sb[0:1, lo : lo + 256],
                                    start=False,
                                    stop=True,
                                )
                        nc.vector.tensor_copy(
                            otile[:, 512 * nk : 512 * (nk + 1)], ps[:]
                        )
                    for tt in range(WT):
                        nc.sync.dma_start(
                            out_v[ih, iw, tt], otile[64 * tt : 64 * (tt + 1), :]
                        )

    _split_drain_waits(nc, mybir)
    return nc


def _get_nc(has_qkvb, has_projb):
    key = (has_qkvb, has_projb)
    if key not in _BUILD_CACHE:
        _BUILD_CACHE[key] = _build(has_qkvb, has_projb)
    return _BUILD_CACHE[key]


def kernel(x, qkv_w, qkv_b, proj_w, proj_b, t, h, w, **_unused):
    from concourse.bass_utils import run_bass_kernel_spmd

    x = np.asarray(x, dtype=np.float32)
    qkv_w = np.asarray(qkv_w, dtype=np.float32)
    qkv_b = np.asarray(qkv_b, dtype=np.float32)
    proj_w = np.asarray(proj_w, dtype=np.float32)
    proj_b = np.asarray(proj_b, dtype=np.float32)
    assert x.shape == (B, N, C), x.shape
    assert int(t) == T and int(h) == H and int(w) == W

    has_qkvb = bool(np.any(qkv_b))
    has_projb = bool(np.any(proj_b))
    nc = _get_nc(has_qkvb, has_projb)

    wqkvT = np.ascontiguousarray(qkv_w.T)
    projT = np.ascontiguousarray(proj_w.T)

    in_maps = []
    for core in range(NCORES):
        b, it = divmod(core, T // WT)
        im = {
            "xs": np.ascontiguousarray(x[b, it * SLAB : (it + 1) * SLAB, :]),
            "wqkvT": wqkvT,
            "projT": projT,
            "ident": np.eye(128, dtype=np.float32),
        }
        if has_qkvb:
            im["qkvb"] = qkv_b.reshape(1, 3 * C)
        if has_projb:
            im["projb"] = proj_b.reshape(1, C)
        in_maps.append(im)

    res = run_bass_kernel_spmd(nc, in_maps, core_ids=list(range(NCORES)))

    y = np.empty((B, N, C), dtype=np.float32)
    for core in range(NCORES):
        b, it = divmod(core, T // WT)
        y[b, it * SLAB : (it + 1) * SLAB, :] = res.results[core]["out"]
    return y



# revision 3
# speedup vs baseline: 1.4283x; 1.4283x over previous
"""Trainium2 Bass kernel for windowed (block-diagonal) multi-head video attention.

Problem: x:[2,8192,1024] -> qkv proj -> 3D-window (2,8,8) attention over a
(8,32,32) token grid, 16 heads x 64 dim -> out proj -> [2,8192,1024].

Sharding: 8 cores, data-parallel over (batch, t-window-group): core (b, it)
owns the 2048-token slab with t in {2it, 2it+1} = 16 independent 128-token
windows.

v2 (bf16): all matmuls in bf16 (1 cyc/row at any ap-size vs f32r's 4x penalty
below ap 256).  x is pre-transposed AND window-permuted on the host, so the
kernel starts from x^T [c, tok] in HBM: no gather DMAs, no PE transposes, no
DVE transpose evictions.  The softmax scale is folded into the Q weights on
the host.  Attention runs as a flat stream over the 16 (window, head-bank)
units per window-group with a lag-2 software pipeline so the single-lane
ln/exp reciprocal chain (ACT) hides behind PE work.  Weights/x: bf16, psum
f32, output bf16 (converted to f32 on host).
"""

import sys

for _p in ("/opt/trn_rl_repo",):
    if _p not in sys.path:
        sys.path.insert(0, _p)

import numpy as np

B, T, H, W = 2, 8, 32, 32
C, NH, HD = 1024, 16, 64
WT, WH, WW = 2, 8, 8
N = T * H * W              # 8192 tokens
SCALE = HD ** -0.5
NCORES = 8
SLAB = N // (T // WT)      # 2048 tokens per (b, it) slab
NWIN = (H // WH) * (W // WW)   # 16 windows per slab
M = WT * WH * WW           # 128 tokens per window
KC = C // 128              # 8 contraction chunks
GW = 4                     # windows per group
TOKG = M * GW              # 512 tokens per group
NGRP = NWIN // GW          # 4 groups

_BUILD_CACHE = {}


def _split_drain_waits(nc, mybir, cap=1, event_cap=2):
    """This walrus build accepts only one sem wait per TPB instruction
    (Tile's scheduler attaches up to 3).  Move the excess onto
    InstEventSemaphore carriers (which hold 2) inserted right before the
    over-subscribed instruction on the same engine — the engine blocks on the
    carriers first, so semantics are unchanged."""
    for f in nc.m.functions:
        for bb in f.blocks:
            i = 0
            while i < len(bb.instructions):
                ins = bb.instructions[i]
                si = ins.sync_info
                my_cap = (
                    event_cap
                    if type(ins).__name__ == "InstEventSemaphore"
                    else cap
                )
                if si is not None and si.on_wait and len(si.on_wait) > my_cap:
                    waits = list(si.on_wait)
                    si.on_wait = waits[:my_cap]
                    extra = waits[my_cap:]
                    carriers = []
                    while extra:
                        chunk, extra = extra[:event_cap], extra[event_cap:]
                        ev = mybir.InstEventSemaphore(
                            name=f"I-{nc.next_id()}-waitsplit", ins=[], outs=[]
                        )
                        ev.engine = ins.engine
                        ev.sync_info = mybir.SyncInfo(
                            on_wait=list(chunk), on_update=[]
                        )
                        nc.register_instruction(ev)
                        carriers.append(ev)
                    bb.instructions[i:i] = carriers
                    i += len(carriers)
                i += 1


def _build(has_qkvb, has_projb):
    import concourse.bass as bass
    import concourse.tile as tile
    from concourse import mybir
    f32 = mybir.dt.float32
    bf = mybir.dt.bfloat16
    Exp = mybir.ActivationFunctionType.Exp
    Ln = mybir.ActivationFunctionType.Ln

    nc = bass.Bass("TRN2", target_bir_lowering=False, debug=False)
    # x^T, window-permuted on host: [p, kc, tok] with tok in (group, win, m)
    # order; c = kc*128 + p
    xT_d = nc.dram_tensor("xT", [128, KC, SLAB], bf, kind="ExternalInput")
    # qkv weights pre-transposed + SCALE folded into Q, blocked by 128-out-col
    # chunks so each DMA is [128, KC*128] with 2KB/partition lines
    wq_d = nc.dram_tensor("wqkvT", [3 * KC, 128, KC, 128], bf,
                          kind="ExternalInput")
    wp_d = nc.dram_tensor("projT", [KC, 128, KC, 128], bf,
                          kind="ExternalInput")
    if has_qkvb:
        qkvb = nc.dram_tensor("qkvb", [1, 3 * C], bf, kind="ExternalInput")
    if has_projb:
        projb = nc.dram_tensor("projb", [1, C], bf, kind="ExternalInput")
    out = nc.dram_tensor("out", [SLAB, C], bf, kind="ExternalOutput")
    # out rows are window-major: row = 512*g + 128*w + m
    out_v = out.ap().rearrange("(g w m) c -> g w m c", g=NGRP, w=GW)

    with tile.TileContext(nc) as tc:
        with (
            tc.tile_pool(name="const", bufs=1) as const_pool,
            tc.tile_pool(name="wq", bufs=1) as wq_pool,
            tc.tile_pool(name="wp", bufs=1) as wp_pool,
            tc.tile_pool(name="xs", bufs=1) as xs_pool,
            tc.tile_pool(name="qkt", bufs=1) as qkt_pool,
            tc.tile_pool(name="v65", bufs=1) as v_pool,
            tc.tile_pool(name="ow", bufs=1) as ow_pool,
            tc.tile_pool(name="E", bufs=3) as e_pool,
            tc.tile_pool(name="rR", bufs=6) as r_pool,
            tc.tile_pool(name="o", bufs=3) as o_pool,
            tc.tile_pool(name="psA", bufs=3, space="PSUM") as psA,
            tc.tile_pool(name="psV", bufs=3, space="PSUM") as psVp,
            tc.tile_pool(name="psS", bufs=2, space="PSUM") as psSp,
        ):
            # ---- constants ----
            ones_colf = const_pool.tile([128, NWIN * GW], f32)
            nc.vector.memset(ones_colf[:], 1.0)
            ones_col = const_pool.tile([128, GW * NH], bf)
            with nc.allow_low_precision(reason="bf16 const"):
                nc.scalar.copy(ones_col[:], ones_colf[:, 0 : GW * NH])
            ones64f = const_pool.tile([1, 64], f32)
            nc.vector.memset(ones64f[:], 1.0)
            ones64 = const_pool.tile([1, 64], bf)
            with nc.allow_low_precision(reason="bf16 const"):
                nc.scalar.copy(ones64[:], ones64f[:])
            if has_qkvb or has_projb:
                onestf = const_pool.tile([1, TOKG], f32)
                nc.vector.memset(onestf[:], 1.0)
                onest = const_pool.tile([1, TOKG], bf)
                with nc.allow_low_precision(reason="bf16 const"):
                    nc.scalar.copy(onest[:], onestf[:])
            if has_qkvb:
                qkvb_sb = const_pool.tile([1, 3 * C], bf)
                nc.sync.dma_start(qkvb_sb[:], qkvb.ap())
            if has_projb:
                projb_sb = const_pool.tile([1, C], bf)
                nc.sync.dma_start(projb_sb[:], projb.ap())

            # ---- resident tensors ----
            xT = xs_pool.tile([128, KC, SLAB], bf)
            wq_sb = wq_pool.tile([128, KC, 3 * C], bf)
            wp_sb = wp_pool.tile([128, KC, C], bf)
            # qkT slots: Q head h -> slot h; K head h -> slot 16+h
            qkT = qkt_pool.tile([64, 4 * KC, TOKG], bf)
            v65 = v_pool.tile([128, GW, NH, HD + 1], bf)
            owT = ow_pool.tile([128, KC, TOKG], bf)

            # DMA order: group-0 x first, then QK weights, then the rest
            for k in range(KC):
                nc.scalar.dma_start(
                    xT[:, k, 0:TOKG], xT_d.ap()[:, k, 0:TOKG]
                )
            for blk in range(16):
                nc.sync.dma_start(
                    wq_sb[:, :, 128 * blk : 128 * (blk + 1)], wq_d.ap()[blk]
                )
            for g in range(1, NGRP):
                for k in range(KC):
                    nc.scalar.dma_start(
                        xT[:, k, TOKG * g : TOKG * (g + 1)],
                        xT_d.ap()[:, k, TOKG * g : TOKG * (g + 1)],
                    )
            for blk in range(16, 24):
                nc.sync.dma_start(
                    wq_sb[:, :, 128 * blk : 128 * (blk + 1)], wq_d.ap()[blk]
                )
            for blk in range(KC):
                nc.sync.dma_start(
                    wp_sb[:, :, 128 * blk : 128 * (blk + 1)], wp_d.ap()[blk]
                )
            # ones column of v65 (written once; V evictions never touch it)
            nc.scalar.copy(
                v65[:, :, :, HD : HD + 1],
                ones_col[:].rearrange("p (g h) -> p g h", g=GW)[:, :, :, None],
            )

            def emit_S(g, w, hbs):
                """S^T = K_h^T.T @ Q_h^T per head; psS [128 k, 4x128 q]."""
                banks = []
                for hb in hbs:
                    psS = psSp.tile([128, 512], f32, tag="psS")
                    for m in range(4):
                        h = 4 * hb + m
                        nc.tensor.matmul(
                            psS[:, 128 * m : 128 * (m + 1)],
                            qkT[:, 16 + h, 128 * w : 128 * (w + 1)],
                            qkT[:, h, 128 * w : 128 * (w + 1)],
                            start=True,
                            stop=True,
                        )
                    banks.append(psS)
                return banks

            def emit_proj(g, w):
                otile = o_pool.tile([128, C], bf, tag="o")
                for nk in range(2):
                    ps = psA.tile([128, 512], f32, tag="psA")
                    for k in range(KC):
                        nc.tensor.matmul(
                            ps[:],
                            owT[:, k, 128 * w : 128 * (w + 1)],
                            wp_sb[:, k, 512 * nk : 512 * (nk + 1)],
                            start=(k == 0),
                            stop=(k == KC - 1 and not has_projb),
                        )
                    if has_projb:
                        nc.tensor.matmul(
                            ps[:],
                            onest[0:1, 0:128],
                            projb_sb[0:1, 512 * nk : 512 * (nk + 1)],
                            start=False,
                            stop=True,
                        )
                    with nc.allow_low_precision(reason="bf16 out"):
                        nc.vector.tensor_copy(
                            otile[:, 512 * nk : 512 * (nk + 1)], ps[:]
                        )
                nc.sync.dma_start(out_v[g, w], otile[:])

            def emit_back(st):
                """Normalization back-end for stream unit st=(psV, g, w, hb)."""
                psV, g, w, hb = st
                L = r_pool.tile([1, 512], f32, tag="r")
                nc.scalar.activation(L[:], psV[64:65, :], Ln)
                r = r_pool.tile([1, 512], bf, tag="r")
                with nc.allow_low_precision(reason="bf16 softmax recip"):
                    nc.scalar.activation(r[:], L[:], Exp, scale=-1.0)
                Rp = psA.tile([64, 512], f32, tag="psA")
                nc.tensor.matmul(Rp[:], ones64[:], r[:], start=True, stop=True)
                # DVE can't read two PSUM srcs; stage the broadcast in SBUF
                R = r_pool.tile([64, 512], f32, tag="R")
                nc.vector.tensor_copy(R[:], Rp[:])
                # heads m=(0..3) at col blocks; m even -> partitions 0:64 of
                # owT, m odd -> 64:128; kc slot = 2*hb + (m>=2)
                psVv = psV[:].rearrange("p (s two m) -> p s two m", two=2, m=128)
                Rv = R[:].rearrange("p (s two m) -> p s two m", two=2, m=128)
                for par in range(2):
                    with nc.allow_low_precision(reason="bf16 attn out"):
                        nc.vector.tensor_tensor(
                            owT[
                                64 * par : 64 * (par + 1),
                                2 * hb : 2 * hb + 2,
                                128 * w : 128 * (w + 1),
                            ],
                            psVv[0:64, :, par, :],
                            Rv[:, :, par, :],
                            op=mybir.AluOpType.mult,
                        )

            backlog = []   # stream units awaiting back-end, lag 2
            projlog = []   # windows whose back-ends all retired

            def drain_one():
                if backlog:
                    st = backlog.pop(0)
                    emit_back(st)
                    if st[3] == 3:
                        projlog.append((st[1], st[2]))

            def drain_proj():
                while projlog:
                    g, w = projlog.pop(0)
                    emit_proj(g, w)

            for g in range(NGRP):
                # ---- QKV projection for this group's 512 tokens ----
                for c in range(16):       # 8 Q chunks then 8 K chunks
                    ps = psA.tile([128, 512], f32, tag="psA")
                    for k in range(KC):
                        nc.tensor.matmul(
                            ps[:],
                            wq_sb[:, k, 128 * c : 128 * (c + 1)],
                            xT[:, k, TOKG * g : TOKG * (g + 1)],
                            start=(k == 0),
                            stop=(k == KC - 1 and not has_qkvb),
                        )
                    if has_qkvb:
                        nc.tensor.matmul(
                            ps[:],
                            qkvb_sb[0:1, 128 * c : 128 * (c + 1)],
                            onest[0:1, :],
                            start=False,
                            stop=True,
                        )
                    with nc.allow_low_precision(reason="bf16 qk evict"):
                        nc.vector.tensor_copy(qkT[:, 2 * c, :], ps[0:64, :])
                        nc.scalar.copy(qkT[:, 2 * c + 1, :], ps[64:128, :])
                    # interleave leftover back-ends from the previous group
                    drain_one()
                drain_proj()
                for tc_ in range(GW):     # V, token-major, per window
                    for nk in range(2):
                        ps = psA.tile([128, 512], f32, tag="psA")
                        for k in range(KC):
                            nc.tensor.matmul(
                                ps[:],
                                xT[
                                    :,
                                    k,
                                    TOKG * g + 128 * tc_ : TOKG * g + 128 * (tc_ + 1),
                                ],
                                wq_sb[:, k, 2 * C + 512 * nk : 2 * C + 512 * (nk + 1)],
                                start=(k == 0),
                                stop=(k == KC - 1 and not has_qkvb),
                            )
                        if has_qkvb:
                            nc.tensor.matmul(
                                ps[:],
                                onest[0:1, 0:128],
                                qkvb_sb[0:1, 2 * C + 512 * nk : 2 * C + 512 * (nk + 1)],
                                start=False,
                                stop=True,
                            )
                        with nc.allow_low_precision(reason="bf16 v evict"):
                            nc.scalar.copy(
                                v65[:, tc_, 8 * nk : 8 * (nk + 1), 0:HD],
                                ps[:].rearrange("p (h e) -> p h e", e=HD),
                            )

                # ---- attention stream: 16 (window, head-bank) units ----
                banks = {}
                for hb, b in zip((0, 1), emit_S(g, 0, (0, 1))):
                    banks[(0, hb)] = b
                for i in range(16):
                    w, hb = divmod(i, 4)
                    psS = banks.pop((w, hb))
                    E = e_pool.tile([128, 512], bf, tag="E")
                    with nc.allow_low_precision(reason="bf16 attn weights"):
                        nc.scalar.activation(E[:], psS[:], Exp)
                    if hb == 1:
                        for hb2, b in zip((2, 3), emit_S(g, w, (2, 3))):
                            banks[(w, hb2)] = b
                    elif hb == 3 and w + 1 < GW:
                        for hb2, b in zip((0, 1), emit_S(g, w + 1, (0, 1))):
                            banks[(w + 1, hb2)] = b
                    psV = psVp.tile([128, 512], f32, tag="psV")
                    for m in range(4):
                        h = 4 * hb + m
                        nc.tensor.matmul(
                            psV[0:65, 128 * m : 128 * (m + 1)],
                            v65[:, w, h, :],
                            E[:, 128 * m : 128 * (m + 1)],
                            start=True,
                            stop=True,
                        )
                    backlog.append((psV, g, w, hb))
                    if len(backlog) > 2:
                        drain_one()
                    drain_proj()

            while backlog:
                drain_one()
            drain_proj()

    _split_drain_waits(nc, mybir)
    return nc


def _get_nc(has_qkvb, has_projb):
    key = (has_qkvb, has_projb)
    if key not in _BUILD_CACHE:
        _BUILD_CACHE[key] = _build(has_qkvb, has_projb)
    return _BUILD_CACHE[key]


def _host_prep(x, qkv_w, qkv_b, proj_w, proj_b):
    """Pre-transpose / permute / cast everything the kernel needs."""
    import ml_dtypes

    bf = ml_dtypes.bfloat16
    qw = np.asarray(qkv_w, np.float32).copy()
    qw[0:C] *= SCALE                       # fold softmax scale into W_q
    # [3C, C] -> [C, 3C] -> [kc, p, 3C] -> oc blocks [24, p, kc, 128]
    wqT = np.ascontiguousarray(qw.T)
    wq_blk = np.ascontiguousarray(
        wqT.reshape(KC, 128, 3 * KC, 128).transpose(2, 1, 0, 3)
    ).astype(bf)
    wpT = np.ascontiguousarray(np.asarray(proj_w, np.float32).T)
    wp_blk = np.ascontiguousarray(
        wpT.reshape(KC, 128, KC, 128).transpose(2, 1, 0, 3)
    ).astype(bf)

    x = np.asarray(x, np.float32)
    # per-core window-permuted x^T: [128 p, kc, 2048 tok]
    xTs = []
    for core in range(NCORES):
        b, it = divmod(core, T // WT)
        slab = x[b, it * SLAB : (it + 1) * SLAB, :]
        # (tt, ih, hh, iw, ww, c) -> (ih, iw, tt, hh, ww, c)
        perm = slab.reshape(WT, 4, WH, 4, WW, C).transpose(1, 3, 0, 2, 4, 5)
        xt = perm.reshape(SLAB, C).T                     # [C, 2048]
        xt = xt.reshape(KC, 128, SLAB).transpose(1, 0, 2)  # [p, kc, tok]
        xTs.append(np.ascontiguousarray(xt).astype(bf))

    qb = np.asarray(qkv_b, np.float32).copy()
    qb[0:C] *= SCALE
    pb = np.asarray(proj_b, np.float32)
    return xTs, wq_blk, wp_blk, qb.astype(bf).reshape(1, 3 * C), \
        pb.astype(bf).reshape(1, C)


def _host_unpermute(rows):
    """[2048, C] window-major bf16 rows -> slab token order f32."""
    a = np.asarray(rows, np.float32)
    # rows are (ih, iw, tt, hh, ww); invert to (tt, ih, hh, iw, ww)
    a = a.reshape(4, 4, WT, WH, WW, C).transpose(2, 0, 3, 1, 4, 5)
    return np.ascontiguousarray(a.reshape(SLAB, C))


def kernel(x, qkv_w, qkv_b, proj_w, proj_b, t, h, w, **_unused):
    from concourse.bass_utils import run_bass_kernel_spmd

    x = np.asarray(x, dtype=np.float32)
    assert x.shape == (B, N, C), x.shape
    assert int(t) == T and int(h) == H and int(w) == W

    qkv_b = np.asarray(qkv_b, dtype=np.float32)
    proj_b = np.asarray(proj_b, dtype=np.float32)
    has_qkvb = bool(np.any(qkv_b))
    has_projb = bool(np.any(proj_b))
    nc = _get_nc(has_qkvb, has_projb)

    xTs, wq_blk, wp_blk, qb, pb = _host_prep(x, qkv_w, qkv_b, proj_w, proj_b)

    in_maps = []
    for core in range(NCORES):
        im = {"xT": xTs[core], "wqkvT": wq_blk, "projT": wp_blk}
        if has_qkvb:
            im["qkvb"] = qb
        if has_projb:
            im["projb"] = pb
        in_maps.append(im)

    res = run_bass_kernel_spmd(nc, in_maps, core_ids=list(range(NCORES)))

    y = np.empty((B, N, C), dtype=np.float32)
    for core in range(NCORES):
        b, it = divmod(core, T // WT)
        y[b, it * SLAB : (it + 1) * SLAB, :] = _host_unpermute(
            res.results[core]["out"]
        )
    return y
